# revision 1
# baseline (speedup 1.0000x reference)
"""Trainium2 Bass kernel for nn_AttentionBasedModulator.

Computes out[b, n, c, h, w] = query_features[b, c, h, w]
                              * support_fibers[c, n] * diag_weight[c]

Sharding: data-parallel over batch B=8, one batch element per NeuronCore.

Per core (b fixed), with HW = H*W = 1024 and channels laid out
channel-major-by-3 (SBUF partition p holds channels 3p, 3p+1, 3p+2):
  - q tile [128, 3, 1024]: one contiguous 1.5 MiB DMA load of q[C, HW].
  - s tile [128, 3*32]: s[c, n] = support_fibers[c, n] * diag_weight[c],
    computed on-chip (3 tiny tensor_scalar ops).
  - per group of n_group prototypes: 3*n_group DVE tensor_scalar
    multiplies (fp32 runs in 2x DVE perf mode) of [128, 1024] into an
    output tile [128, n_group, 3, 1024], then one DMA whose DRAM span is
    fully contiguous (each partition writes one 12 KiB run per prototype).

The kernel is DMA-write bound: ~48 MiB of output per core vs ~1.5 MiB of
input, so everything else overlaps the output-write stream.
"""

import numpy as np

C, NP = 384, 32          # channels, prototypes
B, H, W = 8, 32, 32
HW = H * W
P = 128                  # SBUF partitions
CS = C // P              # channels per partition (3)
N_CORES = 8
N_GROUP = 1              # prototypes per output DMA (tile = N_GROUP*1.5 MiB)
BUFS = 6                 # output tile slots
ACT_SPLIT = 0            # of each group's n_group*CS multiplies, how many go
                         # to the ACT (scalar) engine instead of the DVE


def build(repeat: int = 1, timing: bool = False, n_group: int = N_GROUP,
          bufs: int = BUFS, act_split: int = ACT_SPLIT, dma_lite: bool = False,
          scratch_regions: int = 5, split_q: bool = True,
          dma_pure: bool = False, dual_ring: bool = False,
          fine_edges: bool = True):
    """Build and compile the Bass program for one core.

    timing=True: each repeat writes a distinct Internal DRAM region (so
    stores cannot be dead-store-eliminated); a final DRAM->DRAM readback
    of a few bytes per region forms the only ExternalOutput, so dispatch
    timing is not dominated by fetching 400 MB to the host.
    dma_lite=True: only one multiply per output tile (rest of the tile is
    stale slot data) - isolates DMA-write throughput from DVE work.
    """
    import concourse.bacc as bacc
    import concourse.mybir as mybir
    from concourse.tile import TileContext

    nc = bacc.Bacc(None, target_bir_lowering=False)
    f32 = mybir.dt.float32
    act_copy = mybir.ActivationFunctionType.Copy

    q = nc.dram_tensor("q", [C, HW], f32, kind="ExternalInput")
    sf = nc.dram_tensor("sf", [C, NP], f32, kind="ExternalInput")
    dw = nc.dram_tensor("dw", [C, 1], f32, kind="ExternalInput")
    if timing:
        nreg = min(repeat, scratch_regions)
        scratch = nc.dram_tensor("scratch", [nreg, NP, C, HW], f32,
                                 kind="Internal")
        tiny = nc.dram_tensor("out", [nreg, 4], f32, kind="ExternalOutput")
        out_views = [scratch[r % nreg] for r in range(repeat)]
    else:
        out = nc.dram_tensor("out", [NP, C, HW], f32, kind="ExternalOutput")
        tiny = None
        out_views = [out] * repeat

    # Channel-major-by-3 views: partition p <-> channels 3p..3p+2.
    q_r = q.rearrange("(p cs) f -> p cs f", cs=CS)           # [128, 3, 1024]
    sf_r = sf.rearrange("(p cs) n -> p cs n", cs=CS)         # [128, 3, 32]
    dw_r = dw.rearrange("(p cs) o -> p cs o", cs=CS)         # [128, 3, 1]
    ng = NP // n_group

    with TileContext(nc) as tc:
        with tc.tile_pool(name="consts", bufs=1) as cpool, \
             tc.tile_pool(name="work", bufs=bufs) as wpool:
            # Tiny sf/dw loads first: the s precompute overlaps the q load.
            st = cpool.tile([P, CS, NP], f32, name="st")
            nc.sync.dma_start(out=st[:], in_=sf_r)
            dt_ = cpool.tile([P, CS], f32, name="dt")
            nc.sync.dma_start(out=dt_[:], in_=dw_r)
            qt = cpool.tile([P, CS, HW], f32, name="qt")
            if split_q:
                # Per-cs loads let the first multiplies start ~2 us sooner.
                for cs in range(CS):
                    nc.sync.dma_start(out=qt[:, cs, :], in_=q_r[:, cs, :])
            else:
                nc.sync.dma_start(out=qt[:], in_=q_r)
            for cs in range(CS):
                nc.vector.tensor_scalar_mul(st[:, cs, :], st[:, cs, :],
                                            dt_[:, cs:cs + 1])

            src = None
            if dma_pure:
                # One static source tile, filled once: the repeat loop is
                # pure independent DMA stores (measures the DMA ceiling).
                src = cpool.tile([P, n_group, CS, HW], f32, name="src")
                for j in range(n_group):
                    for cs in range(CS):
                        nc.vector.tensor_scalar_mul(
                            src[:, j, cs, :], qt[:, cs, :], st[:, cs, j:j + 1])

            for r in range(repeat):
                out_r = out_views[r].rearrange(
                    "(ng g) (p cs) f -> ng p g cs f", g=n_group, cs=CS)
                for g in range(ng):
                    dma_eng = nc.scalar if (dual_ring and g % 2) else nc.sync
                    if dma_pure:
                        dma_eng.dma_start(out=out_r[g], in_=src[:])
                        continue
                    ot = wpool.tile([P, n_group, CS, HW], f32, name="ot",
                                    tag="ot")
                    k = 0
                    for j in range(n_group):
                        n = g * n_group + j
                        for cs in range(CS):
                            if dma_lite and k > 0:
                                k += 1
                                continue
                            if k < act_split:
                                nc.scalar.activation(
                                    ot[:, j, cs, :], qt[:, cs, :], act_copy,
                                    scale=st[:, cs, n:n + 1])
                            else:
                                nc.vector.tensor_scalar_mul(
                                    ot[:, j, cs, :], qt[:, cs, :],
                                    st[:, cs, n:n + 1])
                            k += 1
                    if fine_edges and g in (0, ng - 1) and not dma_lite:
                        # Fill/drain the pipeline in 512 KB steps at the
                        # kernel edges: the first DMA starts after one
                        # multiply instead of three, and the final drain is
                        # a third as long.
                        for cs in range(CS):
                            dma_eng.dma_start(out=out_r[g][:, :, cs, :],
                                              in_=ot[:, :, cs, :])
                    else:
                        dma_eng.dma_start(out=out_r[g], in_=ot[:])

            if timing:
                nc.sync.dma_start(out=tiny[:], in_=scratch[:, 0, 0, 0:4])

    nc.compile()
    return nc


def make_in_maps(support_fibers, query_features, diag_weight):
    qf = np.ascontiguousarray(
        np.asarray(query_features, dtype=np.float32).reshape(B, C, HW))
    sfm = np.ascontiguousarray(np.asarray(support_fibers, dtype=np.float32))
    dwm = np.ascontiguousarray(
        np.asarray(diag_weight, dtype=np.float32).reshape(C, 1))
    return [{"q": qf[b], "sf": sfm, "dw": dwm} for b in range(B)]


_state = {}


def _ensure_exec():
    """Build the Bass program once and wrap it in a reusable jitted SPMD
    callable (same ``bass_exec`` primitive / NEFF as
    ``bass_utils.run_bass_kernel_spmd``, which re-traces and re-uploads
    402 MB of zero output buffers on every call). The donated zero output
    buffers are created on-device by a separate tiny jit."""
    if "exec" in _state:
        return
    import jax
    import jax.numpy as jnp
    from jax.experimental.shard_map import shard_map
    from jax.sharding import Mesh, NamedSharding, PartitionSpec

    import concourse.mybir as mybir
    from concourse import bass2jax

    nc = build()
    bass2jax.install_neuronx_cc_hook()

    partition_name = nc.partition_id_tensor.name if nc.partition_id_tensor else None
    in_names, out_names, out_avals = [], [], []
    for alloc in nc.m.functions[0].allocations:
        if not isinstance(alloc, mybir.MemoryLocationSet):
            continue
        name = alloc.memorylocations[0].name
        if alloc.kind == "ExternalInput":
            if name != partition_name:
                in_names.append(name)
        elif alloc.kind == "ExternalOutput":
            out_names.append(name)
            out_avals.append(jax.core.ShapedArray(
                tuple(alloc.tensor_shape), mybir.dt.np(alloc.dtype)))
    n_params = len(in_names)
    all_in_names = list(in_names) + list(out_names)
    if partition_name is not None:
        all_in_names.append(partition_name)

    def _body(*args):
        operands = list(args)
        if partition_name is not None:
            operands.append(bass2jax.partition_id_tensor())
        return tuple(bass2jax._bass_exec_p.bind(
            *operands,
            out_avals=tuple(out_avals),
            in_names=tuple(all_in_names),
            out_names=tuple(out_names),
            lowering_input_output_aliases=(),
            sim_require_finite=True,
            sim_require_nnan=True,
            nc=nc,
        ))

    devices = jax.devices()[:N_CORES]
    mesh = Mesh(np.asarray(devices), ("core",))
    n_outs = len(out_avals)
    sharded = jax.jit(
        shard_map(_body, mesh=mesh,
                  in_specs=(PartitionSpec("core"),) * (n_params + n_outs),
                  out_specs=(PartitionSpec("core"),) * n_outs,
                  check_rep=False),
        donate_argnums=tuple(range(n_params, n_params + n_outs)),
        keep_unused=True,
    )
    sh = NamedSharding(mesh, PartitionSpec("core"))
    zero_shapes = [(N_CORES * a.shape[0], *a.shape[1:]) for a in out_avals]
    zeros_fn = jax.jit(
        lambda: tuple(jnp.zeros(s, a.dtype)
                      for s, a in zip(zero_shapes, out_avals)),
        out_shardings=(sh,) * n_outs)

    _state.update(nc=nc, exec=sharded, zeros=zeros_fn, sharding=sh,
                  in_names=in_names)


def _fast_call(in_maps):
    from concurrent.futures import ThreadPoolExecutor

    import jax

    in_names = _state["in_names"]
    sh = _state["sharding"]
    concat_in = [
        jax.device_put(
            np.concatenate([np.asarray(m[name]) for m in in_maps], axis=0), sh)
        for name in in_names
    ]
    zeros = _state["zeros"]()
    out = _state["exec"](*concat_in, *zeros)[0]  # [N_CORES*NP, C, HW]
    shards = sorted(out.addressable_shards,
                    key=lambda s: s.index[0].start or 0)
    if len(shards) == N_CORES:
        with ThreadPoolExecutor(N_CORES) as ex:
            parts = list(ex.map(lambda s: np.asarray(s.data), shards))
        return np.concatenate(parts, axis=0)
    return np.asarray(out)


def kernel(support_fibers, query_features, diag_weight):
    in_maps = make_in_maps(support_fibers, query_features, diag_weight)
    try:
        _ensure_exec()
        flat = _fast_call(in_maps)
    except Exception:
        from concourse.bass_utils import run_bass_kernel_spmd

        nc = _state.get("nc")
        if nc is None:
            nc = build()
            _state["nc"] = nc
        res = run_bass_kernel_spmd(nc, in_maps, core_ids=list(range(N_CORES)))
        flat = np.concatenate([res.results[b]["out"] for b in range(B)], axis=0)
    return flat.reshape(B, NP, C, H, W)

